# revision 8
# baseline (speedup 1.0000x reference)
"""Euclidean attention block (gnn message passing) on 8 Trainium2 NeuronCores.

Strategy (receiver-sorted edge sharding):
  - Host: sort edges by receiver; shard contiguously across 8 cores; build
    128-edge tiles with <=32 unique receivers each; upload per-tile one-hot
    slot matrices (S / S^T), transposed rbf (+ones row), sh/cutoff, and
    sender/receiver gather indices.
  - Device phase A (replicated): project node features with block-diagonal
    q/k/v weights (inv + ev heads fused, ev passthrough appended) into DRAM
    tables projQ/projK (bf16) and projV (f32).
  - Device phase B (edge-sharded): per tile, indirect-gather sender rows
    (k_inv|k_ev|ev) + v rows and unique-receiver rows (q_inv|q_ev|ev);
    expand receiver rows to edges with a one-hot matmul; compute the filter
    MLP via a stacked [ones; rbf^T; sq(ev_diff)^T] matmul (bias and the
    per-degree invariant weights folded in); form alpha via bf16 products +
    segmented reduces; scatter via per-head one-hot matmuls weighted by
    cutoff*alpha into a [32, 255] psum, DMA'd to per-tile partial outputs.
  - Host: np.add.reduceat over the (globally receiver-sorted) partial rows.
"""

import sys
import numpy as np

sys.path.insert(0, "/opt/trn_rl_repo")

import ml_dtypes  # noqa: E402

import concourse.bass as bass  # noqa: E402
import concourse.bacc as bacc  # noqa: E402
import concourse.mybir as mybir  # noqa: E402
import concourse.tile as tile  # noqa: E402
from concourse.ap import AP  # noqa: E402
from concourse.bass_utils import run_bass_kernel_spmd  # noqa: E402
from concourse.masks import make_identity  # noqa: E402

# ---- problem constants ----
N, E, F = 10000, 160000, 240
H_INV, D_INV = 4, 60
H_EV, D_EV = 3, 80
RBF = 32
REPEATS = np.array([3, 5, 7])
EV_DIM = 15
OFFS = np.array([0, 3, 8, 15])
NCORES = 8
P = 128
SLOTS = 32
SUP = 4  # tiles per supertile
NT_NODES = (N + P - 1) // P  # 79
NPAD = NT_NODES * P  # 10112
CAUG = 256  # padded augmented feature dim (240 + 15 -> 256)
PW = 495  # proj row width: 240 inv + 240 ev + 15 ev passthrough
FW = 480  # filter width: 240 inv + 240 ev
SR = 48  # stacked rows: 1 ones + 32 rbf + 15 sqT
NH = H_INV + H_EV  # 7 "heads" for Sa

F32, BF16, I32 = mybir.dt.float32, mybir.dt.bfloat16, mybir.dt.int32
BF16_NP = ml_dtypes.bfloat16

_CACHE = {}
_RUN_OPTS = {}
_LAST_RES = None


def _block_diag(W):
    # W [H, D, D] -> [H*D, H*D]
    H, D, _ = W.shape
    out = np.zeros((H * D, H * D), np.float32)
    for h in range(H):
        out[h * D : (h + 1) * D, h * D : (h + 1) * D] = W[h]
    return out


def _tile_shard(recv, lo, hi):
    """Split sorted edges [lo,hi) into tiles of <=P edges, <=SLOTS uniques."""
    tiles = []
    e = lo
    while e < hi:
        start = e
        uniq = []
        slots = []
        last = None
        while e < hi and (e - start) < P:
            rv = recv[e]
            if rv != last:
                if len(uniq) == SLOTS:
                    break
                uniq.append(rv)
                last = rv
            slots.append(len(uniq) - 1)
            e += 1
        tiles.append((start, e, np.asarray(slots, np.int64), np.asarray(uniq, np.int64)))
    return tiles


def _preprocess(inputs):
    recv = np.asarray(inputs["receivers"])
    send = np.asarray(inputs["senders"])
    perm = np.argsort(recv, kind="stable")
    recv_s = recv[perm]
    send_s = send[perm]
    rbf_s = np.asarray(inputs["rbf"])[perm]
    sh_s = np.asarray(inputs["sh_vectors"])[perm]
    cut_s = np.asarray(inputs["cutoffs"])[perm][:, 0]

    epc = E // NCORES
    core_tiles = [_tile_shard(recv_s, c * epc, (c + 1) * epc) for c in range(NCORES)]
    T = max(len(t) for t in core_tiles)
    T = ((T + SUP - 1) // SUP) * SUP

    # ---- phase A arrays (shared) ----
    x = np.asarray(inputs["inv_features"], np.float32)
    ev = np.asarray(inputs["ev_features"], np.float32)
    x_aug = np.zeros((NPAD, CAUG), np.float32)
    x_aug[:N, :F] = x
    x_aug[:N, F : F + EV_DIM] = ev
    # xevT [nt, kk, c*128+n] = x_aug[nt*128+n, c*128+kk]
    x3 = x_aug.reshape(NT_NODES, P, 2, P)  # [nt, n, c, kk]
    xevT = np.ascontiguousarray(x3.transpose(0, 3, 2, 1).reshape(NT_NODES, P, CAUG))

    def _aug_w(W_inv, W_ev, scale_inv=1.0, scale_ev=1.0, passthrough=True):
        Wa = np.zeros((CAUG, PW), np.float32)
        Wa[:F, :F] = _block_diag(W_inv) * scale_inv
        Wa[:F, F : 2 * F] = _block_diag(W_ev) * scale_ev
        if passthrough:
            Wa[F : F + EV_DIM, 2 * F : 2 * F + EV_DIM] = np.eye(EV_DIM)
        return Wa

    Wq = _aug_w(
        np.asarray(inputs["W_q_inv"], np.float32),
        np.asarray(inputs["W_q_ev"], np.float32),
        1.0 / np.sqrt(D_INV),
        1.0 / np.sqrt(D_EV),
    )
    Wk = _aug_w(
        np.asarray(inputs["W_k_inv"], np.float32),
        np.asarray(inputs["W_k_ev"], np.float32),
    )
    Wv = np.zeros((CAUG, F), np.float32)
    Wv[:F, :F] = _block_diag(np.asarray(inputs["W_v_inv"], np.float32))

    # Wf_all [48, 480]: row 0 bias; rows 1:33 rbf weights; rows 33:48 per-m
    # weights = filter weight row of that m's degree.
    Wfi = np.asarray(inputs["W_filt_inv"], np.float32)  # [35, 240]
    Wfe = np.asarray(inputs["W_filt_ev"], np.float32)
    bfi = np.asarray(inputs["b_filt_inv"], np.float32)
    bfe = np.asarray(inputs["b_filt_ev"], np.float32)
    # stacked row layout: 0:32 rbf_T, 32:47 sq_T, 47 ones (bias)
    Wf = np.zeros((SR, FW), np.float32)
    Wf[0:RBF, :F] = Wfi[:RBF]
    Wf[0:RBF, F:] = Wfe[:RBF]
    deg_of_m = np.repeat(np.arange(3), REPEATS)  # [15]
    Wf[RBF : RBF + EV_DIM, :F] = Wfi[RBF + deg_of_m]
    Wf[RBF : RBF + EV_DIM, F:] = Wfe[RBF + deg_of_m]
    Wf[SR - 1, :F] = bfi
    Wf[SR - 1, F:] = bfe

    # ---- per-core phase B arrays ----
    TST = T // SUP
    per_core = []
    combine_ids = []  # per core: node ids, tile idx, slot idx (valid rows)
    for c in range(NCORES):
        tiles = core_tiles[c]
        rbf1T = np.zeros((T, RBF, P), np.float32)
        STm = np.zeros((T, SLOTS, P), np.float32)
        shc = np.zeros((T, P, 16), np.float32)
        sidx = np.zeros((T, P), np.int32)
        ridx = np.zeros((T, SLOTS), np.int32)
        ids, tidx, slidx = [], [], []
        for t, (s0, s1, slots, uniq) in enumerate(tiles):
            ne = s1 - s0
            nu = len(uniq)
            rbf1T[t, :, :ne] = rbf_s[s0:s1].T
            STm[t, slots, np.arange(ne)] = 1.0
            shc[t, :ne, :EV_DIM] = sh_s[s0:s1]
            shc[t, :ne, EV_DIM] = cut_s[s0:s1]
            sidx[t, :ne] = send_s[s0:s1]
            ridx[t, :nu] = uniq
            ids.append(uniq)
            tidx.append(np.full(nu, t))
            slidx.append(np.arange(nu))
        Sm = np.ascontiguousarray(STm.transpose(0, 2, 1))  # [T, P, SLOTS]
        per_core.append(
            {
                "xevT": xevT,
                "Wq": Wq,
                "Wk": Wk,
                "Wv": Wv,
                "Wf": Wf,
                "ones_d": np.ones((1, SUP * P), np.float32),
                "rbf1T": rbf1T,
                "STd": STm.astype(BF16_NP),
                "Sd": Sm,
                "shcut": shc,
                "sidx": np.ascontiguousarray(
                    sidx.reshape(TST, SUP, P).transpose(0, 2, 1)
                ),  # [TST, P, SUP]
                "ridx": np.ascontiguousarray(
                    ridx.reshape(TST, SUP, SLOTS).transpose(0, 2, 1)
                ),  # [TST, SLOTS, SUP]
            }
        )
        combine_ids.append(
            (
                np.concatenate(ids) if ids else np.zeros(0, np.int64),
                np.concatenate(tidx).astype(np.int64) if tidx else np.zeros(0, np.int64),
                np.concatenate(slidx).astype(np.int64) if slidx else np.zeros(0, np.int64),
            )
        )
    return T, per_core, combine_ids


def _build(T):
    """Build the Bass kernel for T tiles per core."""
    TST = T // SUP
    nc = bacc.Bacc()

    xevT = nc.dram_tensor("xevT", [NT_NODES, P, CAUG], F32, kind="ExternalInput")
    Wq_d = nc.dram_tensor("Wq", [CAUG, PW], F32, kind="ExternalInput")
    Wk_d = nc.dram_tensor("Wk", [CAUG, PW], F32, kind="ExternalInput")
    Wv_d = nc.dram_tensor("Wv", [CAUG, F], F32, kind="ExternalInput")
    Wf_d = nc.dram_tensor("Wf", [SR, FW], F32, kind="ExternalInput")
    rbf1T = nc.dram_tensor("rbf1T", [T, RBF, P], F32, kind="ExternalInput")
    ones_d = nc.dram_tensor("ones_d", [1, SUP * P], F32, kind="ExternalInput")
    STd = nc.dram_tensor("STd", [T, SLOTS, P], BF16, kind="ExternalInput")
    Sd = nc.dram_tensor("Sd", [T, P, SLOTS], F32, kind="ExternalInput")
    shcut = nc.dram_tensor("shcut", [T, P, 16], F32, kind="ExternalInput")
    sidx = nc.dram_tensor("sidx", [TST, P, SUP], I32, kind="ExternalInput")
    ridx = nc.dram_tensor("ridx", [TST, SLOTS, SUP], I32, kind="ExternalInput")

    projQ = nc.dram_tensor("projQ", [NPAD, PW], BF16)
    projK = nc.dram_tensor("projK", [NPAD, PW], BF16)
    projV = nc.dram_tensor("projV", [NPAD, F], F32)
    dpart = nc.dram_tensor("dpart", [T, SLOTS, 255], F32, kind="ExternalOutput")

    with tile.TileContext(nc) as tc:
        with tc.tile_pool(name="const", bufs=1) as cp:
            Wq_sb = cp.tile([P, 2, PW], F32)
            nc.sync.dma_start(out=Wq_sb[:], in_=Wq_d.rearrange("(c p) w -> p c w", p=P))
            Wk_sb = cp.tile([P, 2, PW], F32)
            nc.sync.dma_start(out=Wk_sb[:], in_=Wk_d.rearrange("(c p) w -> p c w", p=P))
            Wv_sb = cp.tile([P, 2, F], F32)
            nc.sync.dma_start(out=Wv_sb[:], in_=Wv_d.rearrange("(c p) w -> p c w", p=P))
            Wf_sb = cp.tile([SR, FW], F32)
            nc.sync.dma_start(out=Wf_sb[:], in_=Wf_d[:, :])
            ident = cp.tile([P, P], F32)
            make_identity(nc, ident[:])

            # ---------------- phase A ----------------
            with (
                tc.tile_pool(name="pa_sb", bufs=3) as pa,
                tc.tile_pool(name="pa_ps", bufs=2, space="PSUM") as paps,
            ):
                for nt in range(NT_NODES):
                    xev = pa.tile([P, CAUG], F32, tag="xev")
                    nc.sync.dma_start(out=xev[:], in_=xevT[nt])
                    for (proj, Wsb, width, odt, eng) in (
                        (projQ, Wq_sb, PW, BF16, "act"),
                        (projK, Wk_sb, PW, BF16, "dve"),
                        (projV, Wv_sb, F, F32, "dve"),
                    ):
                        pst = paps.tile([P, width], F32, tag=f"ps{width}")
                        nc.tensor.matmul(
                            out=pst[:],
                            lhsT=xev[:, 0:P],
                            rhs=Wsb[:, 0, :],
                            start=True,
                            stop=False,
                        )
                        nc.tensor.matmul(
                            out=pst[:],
                            lhsT=xev[:, P:CAUG],
                            rhs=Wsb[:, 1, :],
                            start=False,
                            stop=True,
                        )
                        ob = pa.tile([P, width], odt, tag=f"o{width}")
                        if eng == "act":
                            nc.scalar.activation(
                                out=ob[:],
                                in_=pst[:],
                                func=mybir.ActivationFunctionType.Copy,
                            )
                        else:
                            nc.vector.tensor_copy(out=ob[:], in_=pst[:])
                        nc.sync.dma_start(
                            out=proj[nt * P : (nt + 1) * P, :], in_=ob[:]
                        )

            # ---------------- phase B ----------------
            with (
                tc.tile_pool(name="pb", bufs=2) as pb,
                tc.tile_pool(name="ps_qe", bufs=2, space="PSUM") as ps_qe,
                tc.tile_pool(name="ps_fw", bufs=2, space="PSUM") as ps_fw,
                tc.tile_pool(name="ps_sq", bufs=1, space="PSUM") as ps_sq,
                tc.tile_pool(name="ps_sc", bufs=2, space="PSUM") as ps_sc,
            ):
                for st in range(TST):
                    t0 = st * SUP
                    sidx_t = pb.tile([P, SUP], I32, tag="sidx")
                    nc.sync.dma_start(out=sidx_t[:], in_=sidx[st])
                    ridx_t = pb.tile([SLOTS, SUP], I32, tag="ridx")
                    nc.sync.dma_start(out=ridx_t[:], in_=ridx[st])

                    stacked = pb.tile([SR, SUP * P], F32, tag="stacked")
                    nc.sync.dma_start(
                        out=stacked[0:RBF, :].rearrange(
                            "p (b n) -> p b n", b=SUP
                        ),
                        in_=rbf1T[t0 : t0 + SUP].rearrange("b p n -> p b n"),
                    )
                    nc.sync.dma_start(
                        out=stacked[SR - 1 : SR, :], in_=ones_d[:, :]
                    )
                    ST_t = pb.tile([SLOTS, SUP * P], BF16, tag="st")
                    nc.sync.dma_start(
                        out=ST_t[:].rearrange("s (b n) -> s b n", b=SUP),
                        in_=STd[t0 : t0 + SUP].rearrange("b s n -> s b n"),
                    )
                    S_t = pb.tile([P, SUP * SLOTS], F32, tag="s")
                    nc.sync.dma_start(
                        out=S_t[:].rearrange("e (b s) -> e b s", b=SUP),
                        in_=Sd[t0 : t0 + SUP].rearrange("b e s -> e b s"),
                    )
                    shc = pb.tile([P, SUP * 16], F32, tag="shc")
                    nc.sync.dma_start(
                        out=shc[:].rearrange("e (b k) -> e b k", b=SUP),
                        in_=shcut[t0 : t0 + SUP].rearrange("b e k -> e b k"),
                    )

                    kq = pb.tile([P, SUP * PW], BF16, tag="kq")
                    kv = pb.tile([P, SUP * F], F32, tag="kv")
                    qu = pb.tile([SLOTS, SUP * PW], BF16, tag="qu")
                    for b in range(SUP):
                        nc.gpsimd.indirect_dma_start(
                            out=kq[:, b * PW : (b + 1) * PW],
                            out_offset=None,
                            in_=projK[:, :],
                            in_offset=bass.IndirectOffsetOnAxis(
                                ap=sidx_t[:, b : b + 1], axis=0
                            ),
                        )
                        nc.gpsimd.indirect_dma_start(
                            out=kv[:, b * F : (b + 1) * F],
                            out_offset=None,
                            in_=projV[:, :],
                            in_offset=bass.IndirectOffsetOnAxis(
                                ap=sidx_t[:, b : b + 1], axis=0
                            ),
                        )
                        nc.gpsimd.indirect_dma_start(
                            out=qu[:, b * PW : (b + 1) * PW],
                            out_offset=None,
                            in_=projQ[:, :],
                            in_offset=bass.IndirectOffsetOnAxis(
                                ap=ridx_t[:, b : b + 1], axis=0
                            ),
                        )

                    # expand receiver rows to edges
                    qexp = pb.tile([P, SUP * PW], BF16, tag="qexp")
                    for b in range(SUP):
                        qe_ps = ps_qe.tile([P, PW], F32, tag="qe")
                        nc.tensor.matmul(
                            out=qe_ps[:],
                            lhsT=ST_t[:, b * P : (b + 1) * P],
                            rhs=qu[:, b * PW : (b + 1) * PW],
                            start=True,
                            stop=True,
                        )
                        nc.vector.tensor_copy(
                            out=qexp[:, b * PW : (b + 1) * PW], in_=qe_ps[:]
                        )

                    kq_v = kq[:].rearrange("e (b w) -> e b w", b=SUP)
                    qexp_v = qexp[:].rearrange("e (b w) -> e b w", b=SUP)

                    # ev_diff = ev_s - ev_r  [P, SUP, 15] (f32)
                    evd = pb.tile([P, SUP * EV_DIM], F32, tag="evd")
                    nc.vector.tensor_tensor(
                        out=evd[:].rearrange("e (b k) -> e b k", b=SUP),
                        in0=kq_v[:, :, 2 * F : 2 * F + EV_DIM],
                        in1=qexp_v[:, :, 2 * F : 2 * F + EV_DIM],
                        op=mybir.AluOpType.subtract,
                    )
                    sqT_ps = ps_sq.tile([EV_DIM, SUP * P], F32, tag="sq")
                    for b in range(SUP):
                        nc.tensor.transpose(
                            out=sqT_ps[:, b * P : (b + 1) * P],
                            in_=evd[:, b * EV_DIM : (b + 1) * EV_DIM],
                            identity=ident[:],
                        )
                    nc.scalar.activation(
                        out=stacked[RBF : RBF + EV_DIM, :],
                        in_=sqT_ps[:],
                        func=mybir.ActivationFunctionType.Square,
                    )

                    # filter mlp
                    fw = pb.tile([P, SUP * FW], BF16, tag="fw")
                    for b in range(SUP):
                        fw_ps = ps_fw.tile([P, FW], F32, tag="fwps")
                        nc.tensor.matmul(
                            out=fw_ps[:],
                            lhsT=stacked[:, b * P : (b + 1) * P],
                            rhs=Wf_sb[:],
                            start=True,
                            stop=True,
                        )
                        nc.scalar.activation(
                            out=fw[:, b * FW : (b + 1) * FW],
                            in_=fw_ps[:],
                            func=mybir.ActivationFunctionType.Copy,
                        )

                    # products + segmented reduces
                    qkw = pb.tile([P, SUP * FW], BF16, tag="qkw")
                    nc.vector.tensor_tensor(
                        out=qkw[:].rearrange("e (b w) -> e b w", b=SUP),
                        in0=qexp_v[:, :, 0:FW],
                        in1=kq_v[:, :, 0:FW],
                        op=mybir.AluOpType.mult,
                    )
                    qkw2 = pb.tile([P, SUP * FW], BF16, tag="qkw2")
                    nc.vector.tensor_tensor(
                        out=qkw2[:].rearrange("e (b w) -> e b w", b=SUP),
                        in0=qkw[:].rearrange("e (b w) -> e b w", b=SUP),
                        in1=fw[:].rearrange("e (b w) -> e b w", b=SUP),
                        op=mybir.AluOpType.mult,
                    )
                    alph = pb.tile([P, SUP * 8], F32, tag="alph")
                    qkw2_v = qkw2[:].rearrange("e (b w) -> e b w", b=SUP)
                    nc.vector.tensor_reduce(
                        out=alph[:].rearrange("e (b k) -> e b k", b=SUP)[:, :, 0:H_INV],
                        in_=qkw2_v[:, :, 0:F].rearrange(
                            "e b (h j) -> e b h j", h=H_INV
                        ),
                        axis=mybir.AxisListType.X,
                        op=mybir.AluOpType.add,
                    )
                    nc.vector.tensor_reduce(
                        out=alph[:].rearrange("e (b k) -> e b k", b=SUP)[
                            :, :, H_INV : H_INV + H_EV
                        ],
                        in_=qkw2_v[:, :, F : 2 * F].rearrange(
                            "e b (h j) -> e b h j", h=H_EV
                        ),
                        axis=mybir.AxisListType.X,
                        op=mybir.AluOpType.add,
                    )
                    # ca = alpha * cutoff
                    ca = pb.tile([P, SUP * 8], F32, tag="ca")
                    shc_v = shc[:].rearrange("e (b k) -> e b k", b=SUP)
                    nc.vector.tensor_tensor(
                        out=ca[:].rearrange("e (b k) -> e b k", b=SUP)[:, :, 0:NH],
                        in0=alph[:].rearrange("e (b k) -> e b k", b=SUP)[:, :, 0:NH],
                        in1=shc_v[:, :, EV_DIM : EV_DIM + 1].to_broadcast(
                            [P, SUP, NH]
                        ),
                        op=mybir.AluOpType.mult,
                    )
                    # Sa[e, b, h, s] = S[e, b, s] * ca[e, b, h]
                    Sa = pb.tile([P, SUP * NH * SLOTS], F32, tag="sa")
                    S_ap = S_t[:]
                    in0 = AP(
                        S_ap.tensor,
                        S_ap.offset,
                        [
                            S_ap.ap[0],
                            [SLOTS, SUP],
                            [0, NH],
                            [1, SLOTS],
                        ],
                    )
                    ca_ap = ca[:]
                    in1 = AP(
                        ca_ap.tensor,
                        ca_ap.offset,
                        [
                            ca_ap.ap[0],
                            [8, SUP],
                            [1, NH],
                            [0, SLOTS],
                        ],
                    )
                    nc.vector.tensor_tensor(
                        out=Sa[:].rearrange(
                            "e (b h s) -> e b h s", b=SUP, h=NH
                        ),
                        in0=in0,
                        in1=in1,
                        op=mybir.AluOpType.mult,
                    )

                    # scatter
                    for b in range(SUP):
                        sc_ps = ps_sc.tile([SLOTS, 256], F32, tag="sc")
                        for h in range(H_INV):
                            nc.tensor.matmul(
                                out=sc_ps[:, h * D_INV : (h + 1) * D_INV],
                                lhsT=Sa[
                                    :,
                                    (b * NH + h) * SLOTS : (b * NH + h + 1) * SLOTS,
                                ],
                                rhs=kv[:, b * F + h * D_INV : b * F + (h + 1) * D_INV],
                                start=True,
                                stop=True,
                            )
                        for d in range(3):
                            nc.tensor.matmul(
                                out=sc_ps[
                                    :, F + OFFS[d] : F + OFFS[d + 1]
                                ],
                                lhsT=Sa[
                                    :,
                                    (b * NH + H_INV + d)
                                    * SLOTS : (b * NH + H_INV + d + 1)
                                    * SLOTS,
                                ],
                                rhs=shc[:, b * 16 + OFFS[d] : b * 16 + OFFS[d + 1]],
                                start=True,
                                stop=True,
                            )
                        sc_sb = pb.tile([SLOTS, 255], F32, tag="scsb")
                        nc.vector.tensor_copy(out=sc_sb[:], in_=sc_ps[:, 0:255])
                        nc.sync.dma_start(out=dpart[t0 + b], in_=sc_sb[:])
    return nc


def _combine(results, combine_ids):
    ids_all, rows_all = [], []
    for c in range(NCORES):
        ids, tidx, slidx = combine_ids[c]
        if len(ids) == 0:
            continue
        dp = np.asarray(results[c]["dpart"])
        rows_all.append(dp[tidx, slidx])
        ids_all.append(ids)
    ids = np.concatenate(ids_all)
    rows = np.concatenate(rows_all, axis=0)
    starts = np.r_[0, np.flatnonzero(np.diff(ids)) + 1]
    sums = np.add.reduceat(rows, starts, axis=0)
    out = np.zeros((N, 255), np.float32)
    out[ids[starts]] = sums
    d_inv = np.ascontiguousarray(out[:, :F])
    d_ev = np.ascontiguousarray(out[:, F : F + EV_DIM])
    return d_inv, d_ev


def kernel(**inputs):
    global _LAST_RES
    T, per_core, combine_ids = _preprocess(inputs)
    if T not in _CACHE:
        nc = _build(T)
        if not nc.is_finalized():
            nc.finalize()
        _CACHE[T] = nc
    nc = _CACHE[T]
    res = run_bass_kernel_spmd(
        nc, per_core, core_ids=list(range(NCORES)), **_RUN_OPTS
    )
    _LAST_RES = res
    return _combine(res.results, combine_ids)


if __name__ == "__main__":
    import reference  # noqa

    inputs = {k: np.asarray(v) for k, v in reference.setup_inputs().items()}
    exp_inv, exp_ev = [np.asarray(o) for o in reference.reference(**reference.setup_inputs())]
    got_inv, got_ev = kernel(**inputs)
    for name, g, e in (("d_inv", got_inv, exp_inv), ("d_ev", got_ev, exp_ev)):
        err = np.abs(g - e).max() / (np.abs(e).max() + 1e-12)
        print(f"{name}: absmax rel err {err:.3e}")
